# revision 5
# baseline (speedup 1.0000x reference)
"""Trainium2 Bass kernel for nn_MicroSpeech: 2-layer diagonal complex LRU net.

Math: |lam| = exp(-exp(nu)) ~= 0.368, so the recurrence h_t = lam*h_{t-1} + u_t
is an 8-tap FIR to ~3e-4 (well inside the bf16 noise floor), factorized
radix-(4,2):
    h_t = sum_{j=0,1} lam^{4j} * (sum_{k=0..3} lam^k u_{t-4j-k})
Tap pairs are packed along the 128-row contraction dim ([T_k; T_{k+1}] lhsT
against a duplicated+shifted rhs buffer), so each FIR stage is 1-2 full-width
matmuls. selu is decomposed with the *expm1* form
    selu(v) = L*relu(v) + L*A*(exp(min(v,0)) - 1)
storing e' = exp(min(v,0))-1 (zero-centered, so bf16 tap/proj rounding stays
relative; no folded DC constants anywhere).

Schedule: each core processes 8192 frames (+16-frame input halo) through
full-length SBUF column buffers (8224 cols), stage-major: every stage sweeps
the whole core in 512-col chunks, keeping the PE free of sync bubbles so the
HAM clock-gate stays at 8/8. PSUM->SBUF traffic is spread across Vector /
Scalar / GpSimd; output staged bf16.
"""
import os

os.environ.setdefault("MYCRO_LOCAL_CACHE", "1")

import numpy as np

WINDOW = 128
H = 32
O2 = 256
L_TOTAL = 65536
NCORES = 8
F = L_TOTAL // NCORES          # frames per core (8192)
HALO = 16
NCB = F + HALO + 16            # sbuf col-buffer width (slack for shifted dups)
NCHUNK = F // 512              # 16 full chunks per core
XROWS = F + HALO               # per-core padded input frames (8208)

SELU_L = 1.0507009873554805
SELU_A = 1.6732632423543772


# ---------------------------------------------------------------- host precompute
def _build_consts(inp):
    def Trep(mu):
        a, b = np.diag(mu.real), np.diag(mu.imag)
        return np.block([[a, -b], [b, a]])

    def Eproj(C, mu):
        Cr, Ci = C.real, C.imag
        return np.hstack([Cr * mu.real[None, :] - Ci * mu.imag[None, :],
                          -Cr * mu.imag[None, :] - Ci * mu.real[None, :]])

    def layer(br, bi, nu, th):
        br, bi, nu, th = [np.asarray(a, np.float64) for a in (br, bi, nu, th)]
        lam = np.exp(-np.exp(nu) + 1j * np.exp(th))
        gamma = np.sqrt(1.0 - np.abs(lam) ** 2)
        B = (br + 1j * bi) * gamma[:, None]
        return lam, B

    lam1, B1 = layer(inp["b1r"], inp["b1i"], inp["nu1"], inp["th1"])
    lam2, B2 = layer(inp["b2r"], inp["b2i"], inp["nu2"], inp["th2"])
    C1 = np.asarray(inp["c1r"], np.float64) + 1j * np.asarray(inp["c1i"], np.float64)
    C2 = np.asarray(inp["c2r"], np.float64) + 1j * np.asarray(inp["c2i"], np.float64)
    D1 = np.asarray(inp["d1"], np.float64)
    D2 = np.asarray(inp["d2"], np.float64)
    W = np.asarray(inp["mlp_w"], np.float64)
    b = np.asarray(inp["mlp_b"], np.float64)
    LA = SELU_L * SELU_A

    o = {}
    o["u1"] = np.vstack([B1.real, B1.imag]).T                       # (128, 64)
    o["A1_01"] = np.vstack([Trep(lam1 ** 0).T, Trep(lam1 ** 1).T])  # (128, 64)
    o["A1_23"] = np.vstack([Trep(lam1 ** 2).T, Trep(lam1 ** 3).T])
    o["B1"] = np.vstack([Eproj(C1, lam1 ** 0).T,
                         Eproj(C1, lam1 ** 4).T])                   # (128, 32)
    o["D1"] = D1.T                                                  # (128, 32)
    o["mlp"] = np.vstack([SELU_L * W, LA * W])                      # (64, 32)
    o["beta"] = b                                                   # (32,)
    B2s = np.vstack([B2.real, B2.imag])
    o["u2"] = np.hstack([SELU_L * B2s, LA * B2s]).T                 # (64, 64)
    o["A2_01"] = np.vstack([Trep(lam2 ** 0).T, Trep(lam2 ** 1).T])
    o["A2_23"] = np.vstack([Trep(lam2 ** 2).T, Trep(lam2 ** 3).T])
    o["B2"] = np.vstack([Trep(lam2 ** 0).T, Trep(lam2 ** 4).T])     # (128, 64)
    G = np.hstack([SELU_L * D2, LA * D2, C2.real, -C2.imag])        # (256, 128)
    o["P2a"] = G[:128].T
    o["P2b"] = G[128:].T
    return {k: np.asarray(v) for k, v in o.items()}


_BLOB_SPECS = [
    ("ident", 128), ("u1", 64), ("A1_01", 64), ("A1_23", 64),
    ("B1", 32), ("D1", 32), ("mlp", 32), ("u2", 64),
    ("A2_01", 64), ("A2_23", 64), ("B2", 64),
    ("P2a", 128), ("P2b", 128), ("beta", 1),
]
_BLOB_OFF = {}
_c = 0
for _n, _w in _BLOB_SPECS:
    _BLOB_OFF[_n] = _c
    _c += _w
BLOB_COLS = _c


def _pack_blob(consts):
    blob = np.zeros((128, BLOB_COLS), np.float32)
    blob[:, :128] = np.eye(128, dtype=np.float32)
    for name, wdt in _BLOB_SPECS:
        if name == "ident":
            continue
        off = _BLOB_OFF[name]
        if name == "beta":
            blob[:H, off] = consts["beta"]
        else:
            m = consts[name].astype(np.float32)
            blob[: m.shape[0], off: off + m.shape[1]] = m
    return blob


# ---------------------------------------------------------------- bass program
_PROGRAM = None


def _build_program():
    import concourse.bacc as bacc
    import concourse.tile as tile
    from concourse import mybir

    nc = bacc.Bacc(None, target_bir_lowering=False)
    dt = mybir.dt
    AF = mybir.ActivationFunctionType
    ALU = mybir.AluOpType

    xin = nc.declare_dram_parameter("xin", [XROWS, WINDOW], dt.float32, isOutput=False)
    wts_d = nc.declare_dram_parameter("wts", [128, BLOB_COLS], dt.float32, isOutput=False)
    yout = nc.declare_dram_parameter("yout", [O2, F], dt.bfloat16, isOutput=True)

    with tile.TileContext(nc) as tc:
        with (
            tc.tile_pool(name="singles", bufs=1) as singles,
            tc.tile_pool(name="work", bufs=3) as work,
            tc.tile_pool(name="er", bufs=6) as erp,
            tc.tile_pool(name="yop", bufs=6) as yop,
            tc.tile_pool(name="psum", bufs=8, space="PSUM") as psum,
        ):
            wts = singles.tile([128, BLOB_COLS], dt.float32)
            nc.sync.dma_start(out=wts, in_=wts_d[:, :])
            wts_r = singles.tile([128, BLOB_COLS], dt.float32r)
            nc.vector.tensor_copy(out=wts_r, in_=wts)
            wts_b = singles.tile([128, BLOB_COLS], dt.bfloat16)
            nc.gpsimd.tensor_copy(out=wts_b, in_=wts)

            def Wr(name, p=128):
                off = _BLOB_OFF[name]
                w = dict(_BLOB_SPECS)[name]
                return wts_r[:p, off: off + w]

            def Wb(name, p=128):
                off = _BLOB_OFF[name]
                w = dict(_BLOB_SPECS)[name]
                return wts_b[:p, off: off + w]

            beta_ap = wts[0:32, _BLOB_OFF["beta"]: _BLOB_OFF["beta"] + 1]

            xT = singles.tile([128, NCB], dt.float32r)
            U1 = singles.tile([128, NCB], dt.bfloat16)
            P1 = singles.tile([128, NCB], dt.bfloat16)
            CE1 = singles.tile([64, NCB], dt.bfloat16)
            Z2 = singles.tile([128, NCB], dt.bfloat16)
            U2 = singles.tile([128, NCB], dt.bfloat16)
            P2B = singles.tile([128, NCB], dt.bfloat16)

            def mm(out, lhsT, rhs, start, stop):
                nc.tensor.matmul(out, lhsT, rhs, start=start, stop=stop)

            # ---- S0: load + transpose x -> xT (col j of xT = xin row j)
            for bt in range(4):
                s4 = work.tile([128, 2048], dt.float32, tag="s4")
                nc.sync.dma_start(
                    out=s4.rearrange("p (q w) -> p q w", q=16),
                    in_=xin[2048 * bt: 2048 * (bt + 1), :]
                        .rearrange("(q p) w -> p q w", p=128))
                for m in range(4):
                    xps = psum.tile([128, 512], dt.float32, tag="ps")
                    for i in range(4):
                        q = 4 * m + i
                        nc.tensor.transpose(
                            xps[:, 128 * i: 128 * (i + 1)],
                            s4[:, 128 * q: 128 * (q + 1)], wts[:, 0:128])
                    c0 = 2048 * bt + 512 * m
                    if m % 2 == 0:
                        nc.vector.tensor_copy(out=xT[:, c0: c0 + 512], in_=xps)
                    else:
                        nc.scalar.copy(out=xT[:, c0: c0 + 512], in_=xps)
            # tail: frames [8192, 8208)
            s4t = singles.tile([16, 128], dt.float32)
            nc.sync.dma_start(out=s4t, in_=xin[F + HALO - 16: F + HALO, :])
            xpst = psum.tile([128, 16], dt.float32, tag="ps")
            nc.tensor.transpose(xpst, s4t[0:16, 0:128], wts[0:16, 0:16])
            nc.vector.tensor_copy(out=xT[:, F: F + HALO], in_=xpst)

            fullc = [(HALO + 512 * c, HALO + 512 * c + 512) for c in range(NCHUNK)]

            # ---- S1: u1 = B~1 @ x
            for (a, b) in [(0, HALO)] + fullc:
                w = b - a
                u1ps = psum.tile([64, w], dt.float32, tag="ps")
                mm(u1ps, Wr("u1"), xT[:, a:b], True, True)
                nc.vector.tensor_copy(out=U1[0:64, a:b], in_=u1ps)
                nc.gpsimd.tensor_copy(out=U1[64:128, a + 1:b + 1], in_=U1[0:64, a:b])

            # U1[64:128, 0] is u1[-1] = 0 (zero-padded x); A1's partial chunk
            # reads it before the dup-copy chain has written it
            nc.gpsimd.memset(U1[64:128, 0:1], 0.0)

            # ---- S2: A1 stage (4 taps, packed pairs)
            for (a, b) in [(2, HALO)] + fullc:
                w = b - a
                p1ps = psum.tile([64, w], dt.float32, tag="ps")
                mm(p1ps, Wb("A1_01"), U1[:, a:b], True, False)
                mm(p1ps, Wb("A1_23"), U1[:, a - 2:b - 2], False, True)
                nc.vector.tensor_copy(out=P1[0:64, a:b], in_=p1ps)
                nc.gpsimd.tensor_copy(out=P1[64:128, a + 4:b + 4], in_=P1[0:64, a:b])

            # ---- S3+S4: B1 (C-absorbed, 2 taps packed) + D1 -> y1; selu1 -> CE1
            for (a, b) in [(6, HALO)] + fullc:
                w = b - a
                y1ps = psum.tile([32, w], dt.float32, tag="ps")
                mm(y1ps, Wb("B1"), P1[:, a:b], True, False)
                mm(y1ps, Wr("D1"), xT[:, a:b], False, True)
                nc.vector.tensor_scalar_max(out=CE1[0:32, a:b], in0=y1ps,
                                            scalar1=0.0)
                er = erp.tile([32, 512], dt.float32r, tag="er")
                nc.scalar.activation(out=er[:, :w], in_=y1ps, func=AF.Exp)
                nc.gpsimd.tensor_scalar(out=CE1[32:64, a:b], in0=er[:, :w],
                                        scalar1=-1.0, scalar2=0.0,
                                        op0=ALU.add, op1=ALU.min)

            # ---- S5+S6: mlp -> z; selu2 -> Z2 rows 0:64
            for (a, b) in [(6, HALO)] + fullc:
                w = b - a
                zps = psum.tile([32, w], dt.float32, tag="ps")
                mm(zps, Wb("mlp", p=64), CE1[:, a:b], True, True)
                nc.vector.tensor_scalar(out=Z2[0:32, a:b], in0=zps,
                                        scalar1=beta_ap, scalar2=0.0,
                                        op0=ALU.add, op1=ALU.max)
                er = erp.tile([32, 512], dt.float32r, tag="er")
                nc.scalar.activation(out=er[:, :w], in_=zps, func=AF.Exp,
                                     bias=beta_ap)
                nc.gpsimd.tensor_scalar(out=Z2[32:64, a:b], in0=er[:, :w],
                                        scalar1=-1.0, scalar2=0.0,
                                        op0=ALU.add, op1=ALU.min)

            # ---- S7: u2
            for (a, b) in [(6, HALO)] + fullc:
                w = b - a
                u2ps = psum.tile([64, w], dt.float32, tag="ps")
                mm(u2ps, Wb("u2", p=64), Z2[0:64, a:b], True, True)
                nc.vector.tensor_copy(out=U2[0:64, a:b], in_=u2ps)
                nc.gpsimd.tensor_copy(out=U2[64:128, a + 1:b + 1], in_=U2[0:64, a:b])

            # ---- S8: A2 stage
            for (a, b) in [(10, HALO)] + fullc:
                w = b - a
                p2ps = psum.tile([64, w], dt.float32, tag="ps")
                mm(p2ps, Wb("A2_01"), U2[:, a:b], True, False)
                mm(p2ps, Wb("A2_23"), U2[:, a - 2:b - 2], False, True)
                nc.scalar.copy(out=P2B[0:64, a:b], in_=p2ps)
                nc.gpsimd.tensor_copy(out=P2B[64:128, a + 4:b + 4], in_=P2B[0:64, a:b])

            # ---- S9: B2 stage (2 taps packed) -> h2 -> Z2 rows 64:128
            for (a, b) in fullc:
                w = b - a
                h2ps = psum.tile([64, w], dt.float32, tag="ps")
                mm(h2ps, Wb("B2"), P2B[:, a:b], True, True)
                nc.scalar.copy(out=Z2[64:128, a:b], in_=h2ps)

            # ---- S10: projection + store
            for (a, b) in fullc:
                w = b - a
                yo = yop.tile([128, 1024], dt.bfloat16, tag="yo")
                for half in range(2):
                    yps = psum.tile([128, w], dt.float32, tag="ps")
                    mm(yps, Wb("P2a" if half == 0 else "P2b"), Z2[:, a:b],
                       True, True)
                    nc.vector.tensor_copy(
                        out=yo[:, 512 * half: 512 * half + w], in_=yps)
                nc.sync.dma_start(
                    out=yout[:, a - HALO: b - HALO]
                        .rearrange("(h p) w -> p h w", p=128),
                    in_=yo.rearrange("p (h w) -> p h w", h=2))
    nc.finalize()
    return nc


def _get_program():
    global _PROGRAM
    if _PROGRAM is None:
        _PROGRAM = _build_program()
    return _PROGRAM


# ---------------------------------------------------------------- host wrapper
def _make_inmaps(inputs):
    consts = _build_consts(inputs)
    blob = _pack_blob(consts)
    ts = np.asarray(inputs["inputs_timeseries"], np.float32).ravel()
    xf = ts.reshape(-1, WINDOW)                 # frame-major (65536, 128)
    in_maps = []
    for core in range(NCORES):
        lo = core * F - HALO
        xpad = np.zeros((XROWS, WINDOW), np.float32)
        a0 = max(0, -lo)
        xpad[a0:] = xf[max(lo, 0): core * F + F]
        in_maps.append({"xin": xpad, "wts": blob})
    return in_maps


def _enable_axon_trace():
    """Shim the missing antenv.axon_hooks so trace=True works under axon."""
    import sys
    import types

    if "antenv.axon_hooks" not in sys.modules:
        from trn_agent_boot.trn_boot import _ntff_profile_via_ctypes

        mod = types.ModuleType("antenv.axon_hooks")
        state = {"hook": None}
        mod.set_axon_ntff_profile_hook = lambda h: state.__setitem__("hook", h)
        mod.get_axon_ntff_profile_hook = lambda: state["hook"]
        sys.modules["antenv.axon_hooks"] = mod
        try:
            import antenv

            antenv.axon_hooks = mod
        except ImportError:
            pass
        hook = _ntff_profile_via_ctypes("/opt/axon/libaxon_pjrt.so")
        assert hook is not None
        mod.set_axon_ntff_profile_hook(hook)
    import concourse.bass_utils as bu

    bu.upload_artifacts = lambda tmpdir: tmpdir


def run(inputs, trace=False, **trace_kwargs):
    from concourse.bass_utils import run_bass_kernel_spmd

    if trace:
        _enable_axon_trace()
    nc = _get_program()
    in_maps = _make_inmaps(inputs)
    res = run_bass_kernel_spmd(nc, in_maps, list(range(NCORES)), trace=trace,
                               **trace_kwargs)
    out = np.concatenate(
        [np.asarray(r["yout"]).astype(np.float32) for r in res.results], axis=1)
    return out, res


def kernel(**inputs) -> np.ndarray:
    out, _ = run(inputs)
    return out


# revision 6
# speedup vs baseline: 2.8627x; 2.8627x over previous
"""Trainium2 Bass kernel for nn_MicroSpeech: 2-layer diagonal complex LRU net.

Math: |lam| = exp(-exp(nu)) ~= 0.368, so the recurrence h_t = lam*h_{t-1} + u_t
is an 8-tap FIR to ~3e-4 (inside the f32r noise floor), factorized radix-(4,2):
    h_t = sum_{j=0,1} lam^{4j} * (sum_{k=0..3} lam^k u_{t-4j-k})
Each tap is one 64-contraction matmul against a shifted column slice of the
full-length SBUF buffer. selu uses the *expm1* form
    selu(v) = L*relu(v) + L*A*(exp(min(v,0)) - 1)
storing e' = exp(min(v,0))-1 (zero-centered -> low-precision-safe, and no
folded DC constants anywhere).

Schedule: each core processes 8192 frames (+16-frame halo) through full-length
column buffers, stage-major 512-col sweeps. PSUM can only be drained by the
Vector and Scalar engines (GpSimd has no PSUM port and is ~10x slower anyway;
DMA cannot read PSUM), so the 14 per-chunk drain/activation ops alternate
between those two engines per chunk parity (exp only on Scalar, expm1-min only
on Vector). Taps are deliberately UNPACKED (no 128-row pair packing): the
extra matmuls are cheaper than the duplicate+shift copies they replace.
Buffers alias (Z2=xT, U2=U1, P2=P1) to fit SBUF at f32r precision.
"""
import os

os.environ.setdefault("MYCRO_LOCAL_CACHE", "1")

import numpy as np

WINDOW = 128
H = 32
O2 = 256
L_TOTAL = 65536
NCORES = 8
F = L_TOTAL // NCORES          # frames per core (8192)
HALO = 16
NCB = F + HALO + 16            # sbuf col-buffer width
NCHUNK = F // 512              # 16 full chunks per core
XROWS = F + HALO               # per-core padded input frames (8208)

SELU_L = 1.0507009873554805
SELU_A = 1.6732632423543772


# ---------------------------------------------------------------- host precompute
def _build_consts(inp):
    def Trep(mu):
        a, b = np.diag(mu.real), np.diag(mu.imag)
        return np.block([[a, -b], [b, a]])

    def Eproj(C, mu):
        Cr, Ci = C.real, C.imag
        return np.hstack([Cr * mu.real[None, :] - Ci * mu.imag[None, :],
                          -Cr * mu.imag[None, :] - Ci * mu.real[None, :]])

    def layer(br, bi, nu, th):
        br, bi, nu, th = [np.asarray(a, np.float64) for a in (br, bi, nu, th)]
        lam = np.exp(-np.exp(nu) + 1j * np.exp(th))
        gamma = np.sqrt(1.0 - np.abs(lam) ** 2)
        B = (br + 1j * bi) * gamma[:, None]
        return lam, B

    lam1, B1 = layer(inp["b1r"], inp["b1i"], inp["nu1"], inp["th1"])
    lam2, B2 = layer(inp["b2r"], inp["b2i"], inp["nu2"], inp["th2"])
    C1 = np.asarray(inp["c1r"], np.float64) + 1j * np.asarray(inp["c1i"], np.float64)
    C2 = np.asarray(inp["c2r"], np.float64) + 1j * np.asarray(inp["c2i"], np.float64)
    D1 = np.asarray(inp["d1"], np.float64)
    D2 = np.asarray(inp["d2"], np.float64)
    W = np.asarray(inp["mlp_w"], np.float64)
    b = np.asarray(inp["mlp_b"], np.float64)
    LA = SELU_L * SELU_A

    o = {}
    o["u1"] = np.vstack([B1.real, B1.imag]).T                       # (128, 64)
    for k in range(4):
        o[f"A1_{k}"] = Trep(lam1 ** k).T                            # (64, 64)
        o[f"A2_{k}"] = Trep(lam2 ** k).T
    for j in range(2):
        o[f"B1_{j}"] = Eproj(C1, lam1 ** (4 * j)).T                 # (64, 32)
        o[f"B2_{j}"] = Trep(lam2 ** (4 * j)).T                      # (64, 64)
    o["D1"] = D1.T                                                  # (128, 32)
    o["mlp"] = np.vstack([SELU_L * W, LA * W])                      # (64, 32)
    o["beta"] = b                                                   # (32,)
    B2s = np.vstack([B2.real, B2.imag])
    o["u2"] = np.hstack([SELU_L * B2s, LA * B2s]).T                 # (64, 64)
    G = np.hstack([SELU_L * D2, LA * D2, C2.real, -C2.imag])        # (256, 128)
    o["P2a"] = G[:128].T
    o["P2b"] = G[128:].T
    return {k: np.asarray(v) for k, v in o.items()}


_BLOB_SPECS = [
    ("ident", 128), ("u1", 64),
    ("A1_0", 64), ("A1_1", 64), ("A1_2", 64), ("A1_3", 64),
    ("B1_0", 32), ("B1_1", 32), ("D1", 32), ("mlp", 32), ("u2", 64),
    ("A2_0", 64), ("A2_1", 64), ("A2_2", 64), ("A2_3", 64),
    ("B2_0", 64), ("B2_1", 64),
    ("P2a", 128), ("P2b", 128), ("beta", 1),
]
_BLOB_OFF = {}
_c = 0
for _n, _w in _BLOB_SPECS:
    _BLOB_OFF[_n] = _c
    _c += _w
BLOB_COLS = _c


def _pack_blob(consts):
    blob = np.zeros((128, BLOB_COLS), np.float32)
    blob[:, :128] = np.eye(128, dtype=np.float32)
    for name, wdt in _BLOB_SPECS:
        if name == "ident":
            continue
        off = _BLOB_OFF[name]
        if name == "beta":
            blob[:H, off] = consts["beta"]
        else:
            m = consts[name].astype(np.float32)
            blob[: m.shape[0], off: off + m.shape[1]] = m
    return blob


# ---------------------------------------------------------------- bass program
_PROGRAM = None


def _build_program():
    import concourse.bacc as bacc
    import concourse.tile as tile
    from concourse import mybir

    nc = bacc.Bacc(None, target_bir_lowering=False)
    dt = mybir.dt
    AF = mybir.ActivationFunctionType
    ALU = mybir.AluOpType

    xin = nc.declare_dram_parameter("xin", [XROWS, WINDOW], dt.float32, isOutput=False)
    wts_d = nc.declare_dram_parameter("wts", [128, BLOB_COLS], dt.float32, isOutput=False)
    yout = nc.declare_dram_parameter("yout", [O2, F], dt.bfloat16, isOutput=True)

    with tile.TileContext(nc) as tc:
        with (
            tc.tile_pool(name="singles", bufs=1) as singles,
            tc.tile_pool(name="work", bufs=3) as work,
            tc.tile_pool(name="er", bufs=6) as erp,
            tc.tile_pool(name="yop", bufs=6) as yop,
            tc.tile_pool(name="psum", bufs=8, space="PSUM") as psum,
        ):
            wts = singles.tile([128, BLOB_COLS], dt.float32)
            nc.sync.dma_start(out=wts, in_=wts_d[:, :])
            wts_r = singles.tile([128, BLOB_COLS], dt.float32r)
            nc.vector.tensor_copy(out=wts_r, in_=wts)
            wts_b = singles.tile([128, BLOB_COLS], dt.bfloat16)
            nc.scalar.copy(out=wts_b, in_=wts)

            def Wr(name, p=128):
                off = _BLOB_OFF[name]
                w = dict(_BLOB_SPECS)[name]
                return wts_r[:p, off: off + w]

            def Wb(name, p=128):
                off = _BLOB_OFF[name]
                w = dict(_BLOB_SPECS)[name]
                return wts_b[:p, off: off + w]

            beta_ap = wts[0:32, _BLOB_OFF["beta"]: _BLOB_OFF["beta"] + 1]

            # big column buffers; later-phase buffers alias earlier ones whose
            # lifetime has ended (true deps enforce the ordering per column)
            xT = singles.tile([128, NCB], dt.float32r)   # x, then Z2=[c2;e2;h2]
            Z2 = xT
            U1 = singles.tile([64, NCB], dt.float32r)    # u1, then u2
            U2 = U1
            P1 = singles.tile([64, NCB], dt.float32r)    # p1, then p2
            P2B = P1
            CE1 = singles.tile([64, NCB], dt.bfloat16)   # [relu(y1); expm1]

            def mm(out, lhsT, rhs, start, stop):
                nc.tensor.matmul(out, lhsT, rhs, start=start, stop=stop)

            def cp(c, out, in_):
                """psum->sbuf drain, alternating Vector/Scalar by chunk."""
                if c % 2 == 0:
                    nc.vector.tensor_copy(out=out, in_=in_)
                else:
                    nc.scalar.copy(out=out, in_=in_)

            # ---- S0: load + transpose x -> xT (col j of xT = xin row j)
            for bt in range(4):
                s4 = work.tile([128, 2048], dt.float32, tag="s4")
                nc.sync.dma_start(
                    out=s4.rearrange("p (q w) -> p q w", q=16),
                    in_=xin[2048 * bt: 2048 * (bt + 1), :]
                        .rearrange("(q p) w -> p q w", p=128))
                for m in range(4):
                    xps = psum.tile([128, 512], dt.float32, tag="ps")
                    for i in range(4):
                        q = 4 * m + i
                        nc.tensor.transpose(
                            xps[:, 128 * i: 128 * (i + 1)],
                            s4[:, 128 * q: 128 * (q + 1)], wts[:, 0:128])
                    c0 = 2048 * bt + 512 * m
                    cp(m, xT[:, c0: c0 + 512], xps)
            # tail: frames [8192, 8208)
            s4t = singles.tile([16, 128], dt.float32)
            nc.sync.dma_start(out=s4t, in_=xin[F: F + HALO, :])
            xpst = psum.tile([128, 16], dt.float32, tag="ps")
            nc.tensor.transpose(xpst, s4t[0:16, 0:128], wts[0:16, 0:16])
            nc.vector.tensor_copy(out=xT[:, F: F + HALO], in_=xpst)

            fullc = [(HALO + 512 * c, HALO + 512 * c + 512) for c in range(NCHUNK)]

            # ---- S1: u1 = B~1 @ x
            for ci, (a, b) in enumerate([(0, HALO)] + fullc):
                u1ps = psum.tile([64, b - a], dt.float32, tag="ps")
                mm(u1ps, Wr("u1"), xT[:, a:b], True, True)
                cp(ci, U1[:, a:b], u1ps)

            # ---- S2: A1 stage, 4 taps
            for ci, (a, b) in enumerate([(4, HALO)] + fullc):
                p1ps = psum.tile([64, b - a], dt.float32, tag="ps")
                for k in range(4):
                    mm(p1ps, Wr(f"A1_{k}", p=64), U1[:, a - k:b - k],
                       k == 0, k == 3)
                cp(ci, P1[:, a:b], p1ps)

            # ---- S3+S4: B1 (C-absorbed, taps 0/4) + D1 -> y1; selu1 -> CE1
            for ci, (a, b) in enumerate([(8, HALO)] + fullc):
                w = b - a
                y1ps = psum.tile([32, w], dt.float32, tag="ps")
                mm(y1ps, Wr("B1_0", p=64), P1[:, a:b], True, False)
                mm(y1ps, Wr("B1_1", p=64), P1[:, a - 4:b - 4], False, False)
                mm(y1ps, Wr("D1"), xT[:, a:b], False, True)
                if ci % 2 == 0:
                    nc.scalar.activation(out=CE1[0:32, a:b], in_=y1ps,
                                         func=AF.Relu)
                else:
                    nc.vector.tensor_scalar_max(out=CE1[0:32, a:b], in0=y1ps,
                                                scalar1=0.0)
                er = erp.tile([32, 512], dt.float32r, tag="er")
                nc.scalar.activation(out=er[:, :w], in_=y1ps, func=AF.Exp)
                nc.vector.tensor_scalar(out=CE1[32:64, a:b], in0=er[:, :w],
                                        scalar1=-1.0, scalar2=0.0,
                                        op0=ALU.add, op1=ALU.min)

            # ---- S5+S6: mlp -> z; selu2 -> Z2 rows 0:64
            for ci, (a, b) in enumerate([(8, HALO)] + fullc):
                w = b - a
                zps = psum.tile([32, w], dt.float32, tag="ps")
                mm(zps, Wb("mlp", p=64), CE1[:, a:b], True, True)
                if ci % 2 == 0:
                    nc.scalar.activation(out=Z2[0:32, a:b], in_=zps,
                                         func=AF.Relu, bias=beta_ap)
                else:
                    nc.vector.tensor_scalar(out=Z2[0:32, a:b], in0=zps,
                                            scalar1=beta_ap, scalar2=0.0,
                                            op0=ALU.add, op1=ALU.max)
                er = erp.tile([32, 512], dt.float32r, tag="er")
                nc.scalar.activation(out=er[:, :w], in_=zps, func=AF.Exp,
                                     bias=beta_ap)
                nc.vector.tensor_scalar(out=Z2[32:64, a:b], in0=er[:, :w],
                                        scalar1=-1.0, scalar2=0.0,
                                        op0=ALU.add, op1=ALU.min)

            # ---- S7: u2
            for ci, (a, b) in enumerate([(8, HALO)] + fullc):
                u2ps = psum.tile([64, b - a], dt.float32, tag="ps")
                mm(u2ps, Wr("u2", p=64), Z2[0:64, a:b], True, True)
                cp(ci, U2[:, a:b], u2ps)

            # ---- S8: A2 stage, 4 taps
            for ci, (a, b) in enumerate([(12, HALO)] + fullc):
                p2ps = psum.tile([64, b - a], dt.float32, tag="ps")
                for k in range(4):
                    mm(p2ps, Wr(f"A2_{k}", p=64), U2[:, a - k:b - k],
                       k == 0, k == 3)
                cp(ci, P2B[:, a:b], p2ps)

            # ---- S9: B2 stage (taps 0/4) -> h2 -> Z2 rows 64:128
            for ci, (a, b) in enumerate(fullc):
                h2ps = psum.tile([64, b - a], dt.float32, tag="ps")
                mm(h2ps, Wr("B2_0", p=64), P2B[:, a:b], True, False)
                mm(h2ps, Wr("B2_1", p=64), P2B[:, a - 4:b - 4], False, True)
                cp(ci, Z2[64:128, a:b], h2ps)

            # ---- S10: projection + store
            for ci, (a, b) in enumerate(fullc):
                w = b - a
                yo = yop.tile([128, 1024], dt.bfloat16, tag="yo")
                for half in range(2):
                    yps = psum.tile([128, w], dt.float32, tag="ps")
                    mm(yps, Wr("P2a" if half == 0 else "P2b"), Z2[:, a:b],
                       True, True)
                    cp(ci + half, yo[:, 512 * half: 512 * half + w], yps)
                nc.sync.dma_start(
                    out=yout[:, a - HALO: b - HALO]
                        .rearrange("(h p) w -> p h w", p=128),
                    in_=yo.rearrange("p (h w) -> p h w", h=2))
    nc.finalize()
    return nc


def _get_program():
    global _PROGRAM
    if _PROGRAM is None:
        _PROGRAM = _build_program()
    return _PROGRAM


# ---------------------------------------------------------------- host wrapper
def _make_inmaps(inputs):
    consts = _build_consts(inputs)
    blob = _pack_blob(consts)
    ts = np.asarray(inputs["inputs_timeseries"], np.float32).ravel()
    xf = ts.reshape(-1, WINDOW)                 # frame-major (65536, 128)
    in_maps = []
    for core in range(NCORES):
        lo = core * F - HALO
        xpad = np.zeros((XROWS, WINDOW), np.float32)
        a0 = max(0, -lo)
        xpad[a0:] = xf[max(lo, 0): core * F + F]
        in_maps.append({"xin": xpad, "wts": blob})
    return in_maps


def _enable_axon_trace():
    """Shim the missing antenv.axon_hooks so trace=True works under axon."""
    import sys
    import types

    if "antenv.axon_hooks" not in sys.modules:
        from trn_agent_boot.trn_boot import _ntff_profile_via_ctypes

        mod = types.ModuleType("antenv.axon_hooks")
        state = {"hook": None}
        mod.set_axon_ntff_profile_hook = lambda h: state.__setitem__("hook", h)
        mod.get_axon_ntff_profile_hook = lambda: state["hook"]
        sys.modules["antenv.axon_hooks"] = mod
        try:
            import antenv

            antenv.axon_hooks = mod
        except ImportError:
            pass
        hook = _ntff_profile_via_ctypes("/opt/axon/libaxon_pjrt.so")
        assert hook is not None
        mod.set_axon_ntff_profile_hook(hook)
    import concourse.bass_utils as bu

    bu.upload_artifacts = lambda tmpdir: tmpdir


def run(inputs, trace=False, **trace_kwargs):
    from concourse.bass_utils import run_bass_kernel_spmd

    if trace:
        _enable_axon_trace()
    nc = _get_program()
    in_maps = _make_inmaps(inputs)
    res = run_bass_kernel_spmd(nc, in_maps, list(range(NCORES)), trace=trace,
                               **trace_kwargs)
    out = np.concatenate(
        [np.asarray(r["yout"]).astype(np.float32) for r in res.results], axis=1)
    return out, res


def kernel(**inputs) -> np.ndarray:
    out, _ = run(inputs)
    return out
